# revision 10
# baseline (speedup 1.0000x reference)
"""Distributed GQA attention kernel for Trainium2 (8 NeuronCores).

Module: B=4, S=2048, H=576, 9 Q heads / 3 KV heads, HD=64, RoPE, causal
softmax, output projection.

Sharding: core c handles batch c//2 and four 256-row query blocks
({0,3,4,7} for even c, {1,2,5,6} for odd c) -- causal work is balanced at
36 real key-tile units per core (padded to 40 for SPMD uniformity; the
causal mask is applied from per-core mask DATA on the last 4 key-tiles of
each slot). Every core computes its batch's full K/V projection locally.

All matmul operands live at partition base 0 (PE tile-position switches
drain the array and cost ~3us each on HW). Projections still stack two
64-dim heads into 128 PSUM partitions; the upper halves are relocated to
base-0 SBUF tiles by cheap SBUF->SBUF DMAs. bf16 everywhere with f32 PSUM
accumulation. Softmax skips the max subtraction; row sums ride along as a
65th PV output row via ones columns interleaved in V. Constants load to
SBUF once at startup; the steady-state pass issues ~33 DMAs.

The attention inner loop is software-pipelined: QK(kc+1) is emitted
before PV(kc) so the PE never sits behind the exp latency, and the
projection work of the NEXT stage is chopped into closures that are
drip-fed into the PE queue between attention units (the last stage pulls
in the next repetition's stage-0 work).
"""

import sys

if "/opt/trn_rl_repo" not in sys.path:
    sys.path.insert(0, "/opt/trn_rl_repo")

import numpy as np

B, S, H = 4, 2048, 576
NH, NKV, HD = 9, 3, 64
BLK = 256           # query block rows
NBLK = S // BLK     # 8
KT = 128            # key tile rows
EXT = [4, 8, 12, 16]  # padded key-tile extent per block slot
NMASK = 16          # total masked key-tiles per core (= last-4 per slot)
BLOCKS_EVEN = [0, 3, 4, 7]
BLOCKS_ODD = [1, 2, 5, 6]
HK = [128, 128, 128, 128, 64]  # contraction tiles over H=576
NPAIR = 5           # q head pairs: (0,1),(2,3),(4,5),(6,7),(8,-)
PW = [128, 128, 128, 128, 64]  # pair widths

_CACHED = {}


def _build(reps=1):
    from concourse import bacc, bass, mybir, tile

    f32 = mybir.dt.float32
    bf16 = mybir.dt.bfloat16
    AF = mybir.ActivationFunctionType
    ALU = mybir.AluOpType

    nc = bacc.Bacc("TRN2", target_bir_lowering=False, debug=False)

    # ---- per-core inputs (bf16 unless noted) ----
    xs = nc.dram_tensor("xs", [4, 128, 5 * 512], bf16, kind="ExternalInput")
    xqd = nc.dram_tensor("xqd", [128, 5 * 1024], bf16, kind="ExternalInput")
    wq = nc.dram_tensor("wq", [128, 5 * 640], bf16, kind="ExternalInput")
    wk = nc.dram_tensor("wk", [128, 5 * 192], bf16, kind="ExternalInput")
    wv = nc.dram_tensor("wv", [128, 5 * 192], bf16, kind="ExternalInput")
    wo = nc.dram_tensor("wo", [128, 5 * 576], bf16, kind="ExternalInput")
    p2d = nc.dram_tensor("p2d", [128, 128], bf16, kind="ExternalInput")
    coskd = nc.dram_tensor("coskd", [128, S], bf16, kind="ExternalInput")
    sinkd = nc.dram_tensor("sinkd", [128, S], bf16, kind="ExternalInput")
    cosqd = nc.dram_tensor("cosqd", [128, 4 * BLK], bf16, kind="ExternalInput")
    sinqd = nc.dram_tensor("sinqd", [128, 4 * BLK], bf16, kind="ExternalInput")
    mskd = nc.dram_tensor("mskd", [128, NMASK * 3 * BLK], bf16, kind="ExternalInput")
    out = nc.dram_tensor("out", [4 * BLK, H], f32, kind="ExternalOutput")

    with tile.TileContext(nc) as tc:
        with (
            tc.tile_pool(name="consts", bufs=1) as cp,
            tc.tile_pool(name="xstream", bufs=2) as xsp,
            tc.tile_pool(name="kvres", bufs=1) as kvres,
            tc.tile_pool(name="qtp", bufs=1) as qtp,
            tc.tile_pool(name="work", bufs=2) as wp,
            tc.tile_pool(name="expp", bufs=4) as expp,
            tc.tile_pool(name="ctp", bufs=1) as ctp,
            tc.tile_pool(name="outp", bufs=2) as outp,
            tc.tile_pool(name="ps1", bufs=2, space="PSUM") as ps1,
            tc.tile_pool(name="scp", bufs=2, space="PSUM") as scp,
            tc.tile_pool(name="psA", bufs=1, space="PSUM") as psA,
        ):
            # ---- load constants (startup only) ----
            wq_sb = cp.tile([128, 5 * 640], bf16, tag="wq")
            nc.sync.dma_start(wq_sb[:], wq.ap())
            wk_sb = cp.tile([128, 5 * 192], bf16, tag="wk")
            nc.sync.dma_start(wk_sb[:], wk.ap())
            wv_sb = cp.tile([128, 5 * 192], bf16, tag="wv")
            nc.sync.dma_start(wv_sb[:], wv.ap())
            wo_sb = cp.tile([128, 5 * 576], bf16, tag="wo")
            nc.sync.dma_start(wo_sb[:], wo.ap())
            p2_sb = cp.tile([128, 128], bf16, tag="p2")
            nc.sync.dma_start(p2_sb[:], p2d.ap())
            cosk_sb = cp.tile([128, S], bf16, tag="cosk")
            nc.sync.dma_start(cosk_sb[:], coskd.ap())
            sink_sb = cp.tile([128, S], bf16, tag="sink")
            nc.sync.dma_start(sink_sb[:], sinkd.ap())
            cosq_sb = cp.tile([128, 4 * BLK], bf16, tag="cosq")
            nc.sync.dma_start(cosq_sb[:], cosqd.ap())
            sinq_sb = cp.tile([128, 4 * BLK], bf16, tag="sinq")
            nc.sync.dma_start(sinq_sb[:], sinqd.ap())
            msk_sb = cp.tile([128, NMASK * 3 * BLK], bf16, tag="msk")
            nc.sync.dma_start(msk_sb[:], mskd.ap())

            # persistent K/V/Q result tiles
            kTp2 = kvres.tile([128, S], bf16, tag="kTp2", name="kTp2")  # g0|g1
            kTg1 = kvres.tile([HD, S], bf16, tag="kTg1", name="kTg1")  # g1 @ base 0
            kTg2 = kvres.tile([HD, S], bf16, tag="kTg2", name="kTg2")  # g2
            # V: 16 key-tiles x (3 groups x 65 cols: 64 v-dims + ones)
            vch = kvres.tile([128, 16 * 195], bf16, tag="vch", name="vch")
            qT = [
                qtp.tile([128, 4 * BLK], bf16, tag=f"qT{p}", name=f"qT{p}")
                for p in range(NPAIR)
            ]
            qhi = [
                qtp.tile([HD, 4 * BLK], bf16, tag=f"qhi{p}", name=f"qhi{p}")
                for p in range(4)
            ]

            # ones columns of vch (positions g*65+64 within each 195 block)
            for st in range(16):
                vones = vch[:, st * 195 : st * 195 + 195].rearrange(
                    "p (g c) -> p g c", c=65
                )[:, :, 64:65]
                nc.vector.memset(vones, 1.0)

            # trigger the exp ACT-table load during the startup DMA wait
            warm = cp.tile([1, 1], f32, tag="warm")
            nc.scalar.activation(warm[:], p2_sb[0:1, 0:1], AF.Exp)

            holder = {}

            def xq_closure():
                def go():
                    t = qtp.tile([128, 5 * 1024], bf16, tag="xq", name="xq")
                    nc.sync.dma_start(t[:], xqd.ap())
                    holder["xq"] = t

                return [go]

            def kv_closures(ch):
                c0 = ch * 512
                cls = []
                st_h = {}

                def dma_x():
                    t = xsp.tile([128, 5 * 512], bf16, tag="xch", name=f"xch{ch}")
                    nc.sync.dma_start(t[:], xs.ap()[ch])
                    st_h["x"] = t

                cls.append(dma_x)

                def k_chain(which):
                    w = 128 if which == 0 else 64
                    xch = st_h["x"]
                    kps = ps1.tile([128, 512], f32, tag="ps1")
                    for kt, hk in enumerate(HK):
                        nc.tensor.matmul(
                            kps[0:w, :],
                            wk_sb[0:hk, kt * 192 + which * 128 : kt * 192 + which * 128 + w],
                            xch[0:hk, kt * 512 : (kt + 1) * 512],
                            start=(kt == 0),
                            stop=(kt == 4),
                        )
                    st_h[f"kps{which}"] = kps

                def k_copy(which):
                    w = 128 if which == 0 else 64
                    kraw = wp.tile([128, 512], bf16, tag="kraw", name=f"kraw{which}")
                    nc.vector.tensor_copy(kraw[0:w, :], st_h[f"kps{which}"][0:w, :])
                    st_h[f"kraw{which}"] = kraw

                def k_rot(which):
                    w = 128 if which == 0 else 64
                    rps = ps1.tile([128, 512], f32, tag="ps1")
                    nc.tensor.matmul(
                        rps[0:w, :], p2_sb[0:w, 0:w], st_h[f"kraw{which}"][0:w, :],
                        start=True, stop=True,
                    )
                    st_h[f"rps{which}"] = rps

                def k_t1(which):
                    w = 128 if which == 0 else 64
                    t1 = wp.tile([128, 512], bf16, tag="t1")
                    nc.vector.tensor_tensor(
                        t1[0:w, :], st_h[f"kraw{which}"][0:w, :],
                        cosk_sb[0:w, c0 : c0 + 512], ALU.mult,
                    )
                    st_h[f"t1{which}"] = t1

                def k_t2(which):
                    w = 128 if which == 0 else 64
                    t2 = wp.tile([128, 512], bf16, tag="t2")
                    nc.vector.tensor_tensor(
                        t2[0:w, :], st_h[f"rps{which}"][0:w, :],
                        sink_sb[0:w, c0 : c0 + 512], ALU.mult,
                    )
                    st_h[f"t2{which}"] = t2

                def k_add(which):
                    w = 128 if which == 0 else 64
                    dst = kTp2 if which == 0 else kTg2
                    nc.vector.tensor_tensor(
                        dst[0:w, c0 : c0 + 512],
                        st_h[f"t1{which}"][0:w, :],
                        st_h[f"t2{which}"][0:w, :],
                        ALU.add,
                    )
                    if which == 0:
                        # relocate g1 to a base-0 tile (PE needs uniform
                        # operand partition bases)
                        nc.sync.dma_start(
                            kTg1[:, c0 : c0 + 512], kTp2[64:128, c0 : c0 + 512]
                        )

                for which in range(2):
                    cls.append(lambda w_=which: k_chain(w_))
                    cls.append(lambda w_=which: k_copy(w_))
                    cls.append(lambda w_=which: k_rot(w_))
                    cls.append(lambda w_=which: k_t1(w_))
                    cls.append(lambda w_=which: k_t2(w_))
                    cls.append(lambda w_=which: k_add(w_))

                def v_chain(st4):
                    xch = st_h["x"]
                    vps = ps1.tile([128, 192], f32, tag="ps1")
                    for kt, hk in enumerate(HK):
                        nc.tensor.matmul(
                            vps[:],
                            xch[0:hk, kt * 512 + st4 * 128 : kt * 512 + (st4 + 1) * 128],
                            wv_sb[0:hk, kt * 192 : (kt + 1) * 192],
                            start=(kt == 0),
                            stop=(kt == 4),
                        )
                    st_h[f"vps{st4}"] = vps

                def v_copy(st4):
                    st = ch * 4 + st4
                    dst = vch[:, st * 195 : st * 195 + 195].rearrange(
                        "p (g c) -> p g c", c=65
                    )[:, :, 0:64]
                    src = st_h[f"vps{st4}"][:].rearrange("p (g c) -> p g c", c=64)
                    nc.vector.tensor_copy(dst, src)

                for st4 in range(4):
                    cls.append(lambda s_=st4: v_chain(s_))
                    cls.append(lambda s_=st4: v_copy(s_))
                return cls

            def q_closures(j):
                cls = []
                st_h = {}

                def q_chain(p):
                    pw = PW[p]
                    xq_t = holder["xq"]
                    qps = ps1.tile([128, BLK], f32, tag="ps1")
                    for kt, hk in enumerate(HK):
                        nc.tensor.matmul(
                            qps[0:pw, :],
                            wq_sb[0:hk, kt * 640 + p * 128 : kt * 640 + p * 128 + pw],
                            xq_t[0:hk, kt * 1024 + j * BLK : kt * 1024 + (j + 1) * BLK],
                            start=(kt == 0),
                            stop=(kt == 4),
                        )
                    st_h[f"qps{p}"] = qps

                def q_copy(p):
                    pw = PW[p]
                    qraw = wp.tile([128, BLK], bf16, tag="qraw")
                    nc.vector.tensor_copy(qraw[0:pw, :], st_h[f"qps{p}"][0:pw, :])
                    st_h[f"qraw{p}"] = qraw

                def q_rot(p):
                    pw = PW[p]
                    rps = ps1.tile([128, BLK], f32, tag="ps1")
                    nc.tensor.matmul(
                        rps[0:pw, :], p2_sb[0:pw, 0:pw], st_h[f"qraw{p}"][0:pw, :],
                        start=True, stop=True,
                    )
                    st_h[f"rq{p}"] = rps

                def q_t1(p):
                    pw = PW[p]
                    tq1 = wp.tile([128, BLK], bf16, tag="tq1")
                    nc.vector.tensor_tensor(
                        tq1[0:pw, :], st_h[f"qraw{p}"][0:pw, :],
                        cosq_sb[0:pw, j * BLK : (j + 1) * BLK], ALU.mult,
                    )
                    st_h[f"tq1{p}"] = tq1

                def q_t2(p):
                    pw = PW[p]
                    tq2 = wp.tile([128, BLK], bf16, tag="tq2")
                    nc.vector.tensor_tensor(
                        tq2[0:pw, :], st_h[f"rq{p}"][0:pw, :],
                        sinq_sb[0:pw, j * BLK : (j + 1) * BLK], ALU.mult,
                    )
                    st_h[f"tq2{p}"] = tq2

                def q_add(p):
                    pw = PW[p]
                    nc.vector.tensor_tensor(
                        qT[p][0:pw, j * BLK : (j + 1) * BLK],
                        st_h[f"tq1{p}"][0:pw, :],
                        st_h[f"tq2{p}"][0:pw, :],
                        ALU.add,
                    )
                    if pw == 128:
                        nc.sync.dma_start(
                            qhi[p][:, j * BLK : (j + 1) * BLK],
                            qT[p][64:128, j * BLK : (j + 1) * BLK],
                        )

                for p in range(NPAIR):
                    cls.append(lambda p_=p: q_chain(p_))
                    cls.append(lambda p_=p: q_copy(p_))
                    cls.append(lambda p_=p: q_rot(p_))
                    cls.append(lambda p_=p: q_t1(p_))
                    cls.append(lambda p_=p: q_t2(p_))
                    cls.append(lambda p_=p: q_add(p_))
                return cls

            def attn(j, filler):
                def pop(n=1):
                    for _ in range(n):
                        if not filler:
                            return
                        filler.pop(0)()

                ext = EXT[j]
                cts = [
                    ctp.tile([128, BLK], bf16, tag=f"ct{t}", name=f"ct{t}")
                    for t in range(4)
                ]
                cts.append(ctp.tile([HD, BLK], bf16, tag="ct4", name="ct4"))
                KSRC = (None, kTg1, kTg2)

                accs = {}

                def emit_qk(g, kc):
                    sps = scp.tile([KT, 3 * BLK], f32, tag="sc")
                    h0 = 3 * g
                    for i in range(3):
                        h = h0 + i
                        p, half = divmod(h, 2)
                        if g == 0:
                            lhsT = kTp2[0:64, kc * KT : (kc + 1) * KT]
                        else:
                            lhsT = KSRC[g][:, kc * KT : (kc + 1) * KT]
                        if half == 0:
                            rhs = qT[p][0:64, j * BLK : (j + 1) * BLK]
                        else:
                            rhs = qhi[p][:, j * BLK : (j + 1) * BLK]
                        nc.tensor.matmul(
                            sps[:, i * BLK : (i + 1) * BLK], lhsT, rhs,
                            start=True, stop=True,
                        )
                    esb = expp.tile([KT, 3 * BLK], bf16, tag="exp")
                    nc.scalar.activation(esb[:], sps[:], AF.Exp)
                    if kc >= ext - 4:
                        nc.vector.tensor_tensor(
                            esb[:], esb[:],
                            msk_sb[:, kc * 3 * BLK : (kc + 1) * 3 * BLK], ALU.mult,
                        )
                    return esb

                def emit_pv(g, kc, esb):
                    accp, accs1 = accs[g]
                    vslice = vch[:, kc * 195 + g * 65 : kc * 195 + g * 65 + 65]
                    nc.tensor.matmul(
                        accp[:], vslice, esb[:, 0:512],
                        start=(kc == 0), stop=(kc == ext - 1),
                    )
                    nc.tensor.matmul(
                        accs1[:], vslice, esb[:, 512:768],
                        start=(kc == 0), stop=(kc == ext - 1),
                    )

                def emit_norm(g):
                    accp, accs1 = accs[g]
                    rec = wp.tile([65, 3 * BLK], f32, tag="rec")
                    nc.vector.reciprocal(rec[64:65, 0:512], accp[64:65, :])
                    nc.vector.reciprocal(rec[64:65, 512:768], accs1[64:65, :])
                    nc.gpsimd.dma_start(rec[0:1, :], rec[64:65, :])
                    bc = wp.tile([HD, 3 * BLK], f32, tag="bc")
                    nc.gpsimd.partition_broadcast(bc[:], rec[0:1, :])
                    for i in range(3):
                        h = 3 * g + i
                        t, lo = divmod(h, 2)
                        src = (
                            accp[0:HD, i * BLK : (i + 1) * BLK]
                            if i < 2
                            else accs1[0:HD, :]
                        )
                        if lo == 0:
                            nc.vector.tensor_tensor(
                                cts[t][0:HD, :], src, bc[:, i * BLK : (i + 1) * BLK],
                                ALU.mult,
                            )
                        else:
                            stg = wp.tile([HD, BLK], bf16, tag="stg")
                            nc.vector.tensor_tensor(
                                stg[:], src, bc[:, i * BLK : (i + 1) * BLK], ALU.mult
                            )
                            nc.gpsimd.dma_start(cts[t][HD:128, :], stg[:])

                prev = None  # (g, kc, esb)
                units_left = 3 * ext
                for g in range(NKV):
                    accs[g] = (
                        psA.tile([65, 512], f32, tag="accp", name="accp"),
                        psA.tile([65, BLK], f32, tag="accs", name="accs"),
                    )
                    for kc in range(ext):
                        esb = emit_qk(g, kc)
                        if prev is not None:
                            pg, pkc, pesb = prev
                            emit_pv(pg, pkc, pesb)
                            if pkc == ext - 1:
                                emit_norm(pg)
                        n = -(-len(filler) // units_left)  # ceil
                        pop(min(n, 3))
                        units_left -= 1
                        prev = (g, kc, esb)
                pg, pkc, pesb = prev
                emit_pv(pg, pkc, pesb)
                emit_norm(pg)
                pop(2)

                # out projection
                q0 = j * BLK
                for half in range(2):
                    h0c = half * 128
                    pa = ps1.tile([128, 512], f32, tag="ps1")
                    pb = ps1.tile([128, 64], f32, tag="ps1")
                    for t in range(5):
                        rw = 128 if t < 4 else 64
                        lhsT = cts[t][:, h0c : h0c + 128]
                        nc.tensor.matmul(
                            pa[:], lhsT, wo_sb[0:rw, t * 576 : t * 576 + 512],
                            start=(t == 0), stop=(t == 4),
                        )
                        nc.tensor.matmul(
                            pb[:], lhsT, wo_sb[0:rw, t * 576 + 512 : t * 576 + 576],
                            start=(t == 0), stop=(t == 4),
                        )
                    osb = outp.tile([128, H], f32, tag="osb")
                    nc.vector.tensor_copy(osb[:, 0:512], pa[:])
                    nc.vector.tensor_copy(osb[:, 512:576], pb[:])
                    nc.sync.dma_start(out.ap()[q0 + h0c : q0 + h0c + 128, :], osb[:])
                    pop(2)
                # anything left over still belongs before the next stage
                while filler:
                    filler.pop(0)()

            def stage0(rep):
                return xq_closure() + kv_closures(0) + q_closures(0)

            # prologue: stage-0 projections of the first rep
            for c in stage0(0):
                c()
            for rep in range(reps):
                attn(0, kv_closures(1) + q_closures(1))
                attn(1, kv_closures(2) + q_closures(2))
                attn(2, kv_closures(3) + q_closures(3))
                attn(3, stage0(rep + 1) if rep + 1 < reps else [])

    nc.compile()
    return nc


def _get_nc(reps=1):
    key = f"nc{reps}"
    if key not in _CACHED:
        _CACHED[key] = _build(reps=reps)
    return _CACHED[key]


def _make_in_maps(x, cos, sin, mask, Wq, Wk, Wv, Wo):
    import ml_dtypes

    f4 = np.float32
    bf = ml_dtypes.bfloat16
    half = HD // 2
    P = np.zeros((HD, HD), f4)
    for m in range(half):
        P[m + half, m] = -1.0
    for m in range(half, HD):
        P[m - half, m] = 1.0
    P2 = np.zeros((128, 128), f4)
    P2[0:64, 0:64] = P
    P2[64:128, 64:128] = P

    scale = np.float32(1.0 / np.sqrt(HD))
    cosT = np.ascontiguousarray(np.asarray(cos).T.astype(f4))  # [64, S]
    sinT = np.ascontiguousarray(np.asarray(sin).T.astype(f4))
    cosk2 = np.concatenate([cosT, cosT], axis=0)  # [128, S]
    sink2 = np.concatenate([sinT, sinT], axis=0)
    maskT_full = np.ascontiguousarray(np.asarray(mask)[0, 0].T.astype(f4))  # [k, q]

    def pack_planes(W, cols_out):
        C = W.shape[1]
        out_arr = np.zeros((128, 5 * cols_out), f4)
        for kt, hk in enumerate(HK):
            out_arr[0:hk, kt * cols_out : kt * cols_out + C] = W[
                kt * 128 : kt * 128 + hk, :
            ]
        return out_arr

    Wq_pk = pack_planes(np.asarray(Wq).astype(f4), 640)
    Wk_pk = pack_planes(np.asarray(Wk).astype(f4), 192)
    Wv_pk = pack_planes(np.asarray(Wv).astype(f4), 192)
    Wo_a = np.asarray(Wo).astype(f4)
    Wo_pk = np.zeros((128, 5 * 576), f4)
    for t in range(5):
        rw = 128 if t < 4 else 64
        Wo_pk[0:rw, t * 576 : (t + 1) * 576] = Wo_a[t * 128 : t * 128 + rw, :]

    x = np.asarray(x).astype(f4)
    in_maps = []
    for c in range(8):
        b = c // 2
        blocks = BLOCKS_EVEN if c % 2 == 0 else BLOCKS_ODD
        xT = np.ascontiguousarray(x[b].T)  # [H, S]
        xpad = np.zeros((640, S), f4)
        xpad[0:H, :] = xT
        planes = xpad.reshape(5, 128, S)
        xs_arr = np.zeros((4, 128, 5 * 512), f4)
        for ch in range(4):
            for kt in range(5):
                xs_arr[ch, :, kt * 512 : (kt + 1) * 512] = planes[
                    kt, :, ch * 512 : (ch + 1) * 512
                ]
        qcols = np.concatenate(
            [xpad[:, blk * BLK : (blk + 1) * BLK] for blk in blocks], axis=1
        )  # [640, 1024]
        qplanes = qcols.reshape(5, 128, 1024)
        xq_arr = np.zeros((128, 5 * 1024), f4)
        for kt in range(5):
            xq_arr[:, kt * 1024 : (kt + 1) * 1024] = qplanes[kt]

        cosq = np.concatenate(
            [cosk2[:, blk * BLK : (blk + 1) * BLK] for blk in blocks], axis=1
        ) * scale
        sinq = np.concatenate(
            [sink2[:, blk * BLK : (blk + 1) * BLK] for blk in blocks], axis=1
        ) * scale

        # masks tripled along the head axis: one multiply covers the whole
        # [128, 3*BLK] exp tile
        mskc = np.empty((128, NMASK * 3 * BLK), f4)
        for jj, blk in enumerate(blocks):
            for off in range(4):
                kc = 4 * jj + off
                sl = maskT_full[kc * KT : (kc + 1) * KT, blk * BLK : (blk + 1) * BLK]
                m01 = (sl > -1.0).astype(f4)
                for i in range(3):
                    mskc[:, (kc * 3 + i) * BLK : (kc * 3 + i + 1) * BLK] = m01

        in_maps.append(
            {
                "xs": xs_arr.astype(bf),
                "xqd": np.ascontiguousarray(xq_arr).astype(bf),
                "wq": Wq_pk.astype(bf),
                "wk": Wk_pk.astype(bf),
                "wv": Wv_pk.astype(bf),
                "wo": Wo_pk.astype(bf),
                "p2d": P2.astype(bf),
                "coskd": cosk2.astype(bf),
                "sinkd": sink2.astype(bf),
                "cosqd": np.ascontiguousarray(cosq).astype(bf),
                "sinqd": np.ascontiguousarray(sinq).astype(bf),
                "mskd": mskc.astype(bf),
            }
        )
    return in_maps


def kernel(x, cos, sin, mask, Wq, Wk, Wv, Wo, _trace=False, _trace_kwargs=None):
    from concourse import bass_utils

    in_maps = _make_in_maps(
        np.asarray(x), np.asarray(cos), np.asarray(sin), np.asarray(mask),
        np.asarray(Wq), np.asarray(Wk), np.asarray(Wv), np.asarray(Wo),
    )
    nc = _get_nc()
    kw = {}
    if _trace:
        kw["trace"] = True
        if _trace_kwargs:
            kw.update(_trace_kwargs)
    res = bass_utils.run_bass_kernel_spmd(nc, in_maps, core_ids=list(range(8)), **kw)
    out = np.empty((B, S, H), np.float32)
    for c in range(8):
        b = c // 2
        blocks = BLOCKS_EVEN if c % 2 == 0 else BLOCKS_ODD
        o = res.results[c]["out"]  # [1024, 576]
        for j, blk in enumerate(blocks):
            out[b, blk * BLK : (blk + 1) * BLK, :] = o[j * BLK : (j + 1) * BLK, :]
    if _trace:
        _CACHED["last_result"] = res
    return out


# revision 11
# speedup vs baseline: 1.5393x; 1.5393x over previous
"""Distributed GQA attention kernel for Trainium2 (8 NeuronCores).

Module: B=4, S=2048, H=576, 9 Q heads / 3 KV heads, HD=64, RoPE, causal
softmax, output projection.

Sharding: core c handles batch c//2 and four 256-row query blocks
({0,3,4,7} for even c, {1,2,5,6} for odd c) -- causal work is balanced at
36 real key-tile units per core (padded to 40 for SPMD uniformity; the
causal mask is applied from per-core mask DATA on the last 4 key-tiles of
each slot). Every core computes its batch's full K/V projection locally.

All matmul operands live at partition base 0 (PE tile-position switches
drain the array and cost ~3us each on HW). Projections still stack two
64-dim heads into 128 PSUM partitions; the upper halves are relocated to
base-0 SBUF tiles by cheap SBUF->SBUF DMAs. bf16 everywhere with f32 PSUM
accumulation. Softmax skips the max subtraction; row sums ride along as a
65th PV output row via ones columns interleaved in V. Constants load to
SBUF once at startup; the steady-state pass issues ~33 DMAs.

The attention inner loop is software-pipelined: QK(kc+1) is emitted
before PV(kc) so the PE never sits behind the exp latency, and the
projection work of the NEXT stage is chopped into closures that are
drip-fed into the PE queue between attention units (the last stage pulls
in the next repetition's stage-0 work).
"""

import sys

if "/opt/trn_rl_repo" not in sys.path:
    sys.path.insert(0, "/opt/trn_rl_repo")

import numpy as np

B, S, H = 4, 2048, 576
NH, NKV, HD = 9, 3, 64
BLK = 256           # query block rows
NBLK = S // BLK     # 8
KT = 128            # key tile rows
EXT = [4, 8, 12, 16]  # padded key-tile extent per block slot
NMASK = 16          # total masked key-tiles per core (= last-4 per slot)
BLOCKS_EVEN = [0, 3, 4, 7]
BLOCKS_ODD = [1, 2, 5, 6]
HK = [128, 128, 128, 128, 64]  # contraction tiles over H=576
NPAIR = 5           # q head pairs: (0,1),(2,3),(4,5),(6,7),(8,-)
PW = [128, 128, 128, 128, 64]  # pair widths

_CACHED = {}


def _build(reps=1):
    from concourse import bacc, bass, mybir, tile

    f32 = mybir.dt.float32
    bf16 = mybir.dt.bfloat16
    AF = mybir.ActivationFunctionType
    ALU = mybir.AluOpType

    nc = bacc.Bacc("TRN2", target_bir_lowering=False, debug=False)

    # ---- per-core inputs (bf16 unless noted) ----
    xs = nc.dram_tensor("xs", [4, 128, 5 * 512], bf16, kind="ExternalInput")
    xqd = nc.dram_tensor("xqd", [128, 5 * 1024], bf16, kind="ExternalInput")
    wq = nc.dram_tensor("wq", [128, 5 * 640], bf16, kind="ExternalInput")
    wk = nc.dram_tensor("wk", [128, 5 * 192], bf16, kind="ExternalInput")
    wv = nc.dram_tensor("wv", [128, 5 * 192], bf16, kind="ExternalInput")
    wo = nc.dram_tensor("wo", [128, 5 * 576], bf16, kind="ExternalInput")
    p2d = nc.dram_tensor("p2d", [128, 128], bf16, kind="ExternalInput")
    coskd = nc.dram_tensor("coskd", [128, S], bf16, kind="ExternalInput")
    sinkd = nc.dram_tensor("sinkd", [128, S], bf16, kind="ExternalInput")
    cosqd = nc.dram_tensor("cosqd", [128, 4 * BLK], bf16, kind="ExternalInput")
    sinqd = nc.dram_tensor("sinqd", [128, 4 * BLK], bf16, kind="ExternalInput")
    mskd = nc.dram_tensor("mskd", [128, NMASK * 3 * BLK], bf16, kind="ExternalInput")
    out = nc.dram_tensor("out", [4 * BLK, H], f32, kind="ExternalOutput")

    with tile.TileContext(nc) as tc:
        with (
            tc.tile_pool(name="consts", bufs=1) as cp,
            tc.tile_pool(name="xstream", bufs=2) as xsp,
            tc.tile_pool(name="kvres", bufs=1) as kvres,
            tc.tile_pool(name="qtp", bufs=1) as qtp,
            tc.tile_pool(name="work", bufs=2) as wp,
            tc.tile_pool(name="expp", bufs=4) as expp,
            tc.tile_pool(name="ctp", bufs=1) as ctp,
            tc.tile_pool(name="outp", bufs=2) as outp,
            tc.tile_pool(name="ps1", bufs=2, space="PSUM") as ps1,
            tc.tile_pool(name="scp", bufs=2, space="PSUM") as scp,
            tc.tile_pool(name="psA", bufs=1, space="PSUM") as psA,
        ):
            # ---- load constants (startup only) ----
            wq_sb = cp.tile([128, 5 * 640], bf16, tag="wq")
            nc.sync.dma_start(wq_sb[:], wq.ap())
            wk_sb = cp.tile([128, 5 * 192], bf16, tag="wk")
            nc.sync.dma_start(wk_sb[:], wk.ap())
            wv_sb = cp.tile([128, 5 * 192], bf16, tag="wv")
            nc.sync.dma_start(wv_sb[:], wv.ap())
            wo_sb = cp.tile([128, 5 * 576], bf16, tag="wo")
            nc.sync.dma_start(wo_sb[:], wo.ap())
            p2_sb = cp.tile([128, 128], bf16, tag="p2")
            nc.sync.dma_start(p2_sb[:], p2d.ap())
            cosk_sb = cp.tile([128, S], bf16, tag="cosk")
            nc.sync.dma_start(cosk_sb[:], coskd.ap())
            sink_sb = cp.tile([128, S], bf16, tag="sink")
            nc.sync.dma_start(sink_sb[:], sinkd.ap())
            cosq_sb = cp.tile([128, 4 * BLK], bf16, tag="cosq")
            nc.sync.dma_start(cosq_sb[:], cosqd.ap())
            sinq_sb = cp.tile([128, 4 * BLK], bf16, tag="sinq")
            nc.sync.dma_start(sinq_sb[:], sinqd.ap())
            msk_sb = cp.tile([128, NMASK * 3 * BLK], bf16, tag="msk")
            nc.sync.dma_start(msk_sb[:], mskd.ap())

            # persistent K/V/Q result tiles
            kTp2 = kvres.tile([128, S], bf16, tag="kTp2", name="kTp2")  # g0|g1
            kTg1 = kvres.tile([HD, S], bf16, tag="kTg1", name="kTg1")  # g1 @ base 0
            kTg2 = kvres.tile([HD, S], bf16, tag="kTg2", name="kTg2")  # g2
            # V: 16 key-tiles x (3 groups x 65 cols: 64 v-dims + ones)
            vch = kvres.tile([128, 16 * 195], bf16, tag="vch", name="vch")
            qT = [
                qtp.tile([128, 4 * BLK], bf16, tag=f"qT{p}", name=f"qT{p}")
                for p in range(NPAIR)
            ]
            qhi = [
                qtp.tile([HD, 4 * BLK], bf16, tag=f"qhi{p}", name=f"qhi{p}")
                for p in range(4)
            ]

            # ones columns of vch (positions g*65+64 within each 195 block)
            for st in range(16):
                vones = vch[:, st * 195 : st * 195 + 195].rearrange(
                    "p (g c) -> p g c", c=65
                )[:, :, 64:65]
                nc.vector.memset(vones, 1.0)

            # trigger the exp ACT-table load during the startup DMA wait
            warm = cp.tile([1, 1], f32, tag="warm")
            nc.scalar.activation(warm[:], p2_sb[0:1, 0:1], AF.Exp)

            holder = {}

            def xq_closure():
                def go():
                    t = qtp.tile([128, 5 * 1024], bf16, tag="xq", name="xq")
                    nc.sync.dma_start(t[:], xqd.ap())
                    holder["xq"] = t

                return [go]

            def kv_closures(ch):
                c0 = ch * 512
                cls = []
                st_h = {}

                def dma_x():
                    t = xsp.tile([128, 5 * 512], bf16, tag="xch", name=f"xch{ch}")
                    nc.sync.dma_start(t[:], xs.ap()[ch])
                    st_h["x"] = t

                cls.append(dma_x)

                def k_chain(which):
                    w = 128 if which == 0 else 64
                    xch = st_h["x"]
                    kps = ps1.tile([128, 512], f32, tag="ps1")
                    for kt, hk in enumerate(HK):
                        nc.tensor.matmul(
                            kps[0:w, :],
                            wk_sb[0:hk, kt * 192 + which * 128 : kt * 192 + which * 128 + w],
                            xch[0:hk, kt * 512 : (kt + 1) * 512],
                            start=(kt == 0),
                            stop=(kt == 4),
                        )
                    st_h[f"kps{which}"] = kps

                def k_copy(which):
                    w = 128 if which == 0 else 64
                    kraw = wp.tile([128, 512], bf16, tag="kraw", name=f"kraw{which}")
                    nc.vector.tensor_copy(kraw[0:w, :], st_h[f"kps{which}"][0:w, :])
                    st_h[f"kraw{which}"] = kraw

                def k_rot(which):
                    w = 128 if which == 0 else 64
                    rps = ps1.tile([128, 512], f32, tag="ps1")
                    nc.tensor.matmul(
                        rps[0:w, :], p2_sb[0:w, 0:w], st_h[f"kraw{which}"][0:w, :],
                        start=True, stop=True,
                    )
                    st_h[f"rps{which}"] = rps

                def k_t1(which):
                    w = 128 if which == 0 else 64
                    t1 = wp.tile([128, 512], bf16, tag="t1")
                    nc.vector.tensor_tensor(
                        t1[0:w, :], st_h[f"kraw{which}"][0:w, :],
                        cosk_sb[0:w, c0 : c0 + 512], ALU.mult,
                    )
                    st_h[f"t1{which}"] = t1

                def k_t2(which):
                    w = 128 if which == 0 else 64
                    t2 = wp.tile([128, 512], bf16, tag="t2")
                    nc.vector.tensor_tensor(
                        t2[0:w, :], st_h[f"rps{which}"][0:w, :],
                        sink_sb[0:w, c0 : c0 + 512], ALU.mult,
                    )
                    st_h[f"t2{which}"] = t2

                def k_add(which):
                    w = 128 if which == 0 else 64
                    dst = kTp2 if which == 0 else kTg2
                    nc.vector.tensor_tensor(
                        dst[0:w, c0 : c0 + 512],
                        st_h[f"t1{which}"][0:w, :],
                        st_h[f"t2{which}"][0:w, :],
                        ALU.add,
                    )
                    if which == 0:
                        # relocate g1 to a base-0 tile (PE needs uniform
                        # operand partition bases)
                        nc.sync.dma_start(
                            kTg1[:, c0 : c0 + 512], kTp2[64:128, c0 : c0 + 512]
                        )

                for which in range(2):
                    cls.append(lambda w_=which: k_chain(w_))
                    cls.append(lambda w_=which: k_copy(w_))
                    cls.append(lambda w_=which: k_rot(w_))
                    cls.append(lambda w_=which: k_t1(w_))
                    cls.append(lambda w_=which: k_t2(w_))
                    cls.append(lambda w_=which: k_add(w_))

                def v_chain(st4):
                    xch = st_h["x"]
                    vps = ps1.tile([128, 192], f32, tag="ps1")
                    for kt, hk in enumerate(HK):
                        nc.tensor.matmul(
                            vps[:],
                            xch[0:hk, kt * 512 + st4 * 128 : kt * 512 + (st4 + 1) * 128],
                            wv_sb[0:hk, kt * 192 : (kt + 1) * 192],
                            start=(kt == 0),
                            stop=(kt == 4),
                        )
                    st_h[f"vps{st4}"] = vps

                def v_copy(st4):
                    st = ch * 4 + st4
                    dst = vch[:, st * 195 : st * 195 + 195].rearrange(
                        "p (g c) -> p g c", c=65
                    )[:, :, 0:64]
                    src = st_h[f"vps{st4}"][:].rearrange("p (g c) -> p g c", c=64)
                    nc.vector.tensor_copy(dst, src)

                for st4 in range(4):
                    cls.append(lambda s_=st4: v_chain(s_))
                    cls.append(lambda s_=st4: v_copy(s_))
                return cls

            def q_closures(j):
                cls = []
                st_h = {}

                def q_chain(p):
                    pw = PW[p]
                    xq_t = holder["xq"]
                    qps = ps1.tile([128, BLK], f32, tag="ps1")
                    for kt, hk in enumerate(HK):
                        nc.tensor.matmul(
                            qps[0:pw, :],
                            wq_sb[0:hk, kt * 640 + p * 128 : kt * 640 + p * 128 + pw],
                            xq_t[0:hk, kt * 1024 + j * BLK : kt * 1024 + (j + 1) * BLK],
                            start=(kt == 0),
                            stop=(kt == 4),
                        )
                    st_h[f"qps{p}"] = qps

                def q_copy(p):
                    pw = PW[p]
                    qraw = wp.tile([128, BLK], bf16, tag="qraw")
                    nc.vector.tensor_copy(qraw[0:pw, :], st_h[f"qps{p}"][0:pw, :])
                    st_h[f"qraw{p}"] = qraw

                def q_rot(p):
                    pw = PW[p]
                    rps = ps1.tile([128, BLK], f32, tag="ps1")
                    nc.tensor.matmul(
                        rps[0:pw, :], p2_sb[0:pw, 0:pw], st_h[f"qraw{p}"][0:pw, :],
                        start=True, stop=True,
                    )
                    st_h[f"rq{p}"] = rps

                def q_t1(p):
                    pw = PW[p]
                    tq1 = wp.tile([128, BLK], bf16, tag="tq1")
                    nc.vector.tensor_tensor(
                        tq1[0:pw, :], st_h[f"qraw{p}"][0:pw, :],
                        cosq_sb[0:pw, j * BLK : (j + 1) * BLK], ALU.mult,
                    )
                    st_h[f"tq1{p}"] = tq1

                def q_t2(p):
                    pw = PW[p]
                    tq2 = wp.tile([128, BLK], bf16, tag="tq2")
                    nc.vector.tensor_tensor(
                        tq2[0:pw, :], st_h[f"rq{p}"][0:pw, :],
                        sinq_sb[0:pw, j * BLK : (j + 1) * BLK], ALU.mult,
                    )
                    st_h[f"tq2{p}"] = tq2

                def q_add(p):
                    pw = PW[p]
                    nc.vector.tensor_tensor(
                        qT[p][0:pw, j * BLK : (j + 1) * BLK],
                        st_h[f"tq1{p}"][0:pw, :],
                        st_h[f"tq2{p}"][0:pw, :],
                        ALU.add,
                    )
                    if pw == 128:
                        nc.sync.dma_start(
                            qhi[p][:, j * BLK : (j + 1) * BLK],
                            qT[p][64:128, j * BLK : (j + 1) * BLK],
                        )

                for p in range(NPAIR):
                    cls.append(lambda p_=p: q_chain(p_))
                    cls.append(lambda p_=p: q_copy(p_))
                    cls.append(lambda p_=p: q_rot(p_))
                    cls.append(lambda p_=p: q_t1(p_))
                    cls.append(lambda p_=p: q_t2(p_))
                    cls.append(lambda p_=p: q_add(p_))
                return cls

            def attn(j, filler):
                def pop(n=1):
                    for _ in range(n):
                        if not filler:
                            return
                        filler.pop(0)()

                ext = EXT[j]
                cts = [
                    ctp.tile([128, BLK], bf16, tag=f"ct{t}", name=f"ct{t}")
                    for t in range(4)
                ]
                cts.append(ctp.tile([HD, BLK], bf16, tag="ct4", name="ct4"))
                KSRC = (None, kTg1, kTg2)

                accs = {}

                def emit_qk(g, kc):
                    sps = scp.tile([KT, 3 * BLK], f32, tag="sc")
                    h0 = 3 * g
                    for i in range(3):
                        h = h0 + i
                        p, half = divmod(h, 2)
                        if g == 0:
                            lhsT = kTp2[0:64, kc * KT : (kc + 1) * KT]
                        else:
                            lhsT = KSRC[g][:, kc * KT : (kc + 1) * KT]
                        if half == 0:
                            rhs = qT[p][0:64, j * BLK : (j + 1) * BLK]
                        else:
                            rhs = qhi[p][:, j * BLK : (j + 1) * BLK]
                        nc.tensor.matmul(
                            sps[:, i * BLK : (i + 1) * BLK], lhsT, rhs,
                            start=True, stop=True,
                        )
                    esb = expp.tile([KT, 3 * BLK], bf16, tag="exp")
                    nc.scalar.activation(esb[:], sps[:], AF.Exp)
                    if kc >= ext - 4:
                        nc.vector.tensor_tensor(
                            esb[:], esb[:],
                            msk_sb[:, kc * 3 * BLK : (kc + 1) * 3 * BLK], ALU.mult,
                        )
                    return esb

                def emit_pv(g, kc, esb):
                    accp, accs1 = accs[g]
                    vslice = vch[:, kc * 195 + g * 65 : kc * 195 + g * 65 + 65]
                    nc.tensor.matmul(
                        accp[:], vslice, esb[:, 0:512],
                        start=(kc == 0), stop=(kc == ext - 1),
                    )
                    nc.tensor.matmul(
                        accs1[:], vslice, esb[:, 512:768],
                        start=(kc == 0), stop=(kc == ext - 1),
                    )

                def emit_norm(g):
                    accp, accs1 = accs[g]
                    rec = wp.tile([65, 3 * BLK], f32, tag="rec")
                    nc.vector.reciprocal(rec[64:65, 0:512], accp[64:65, :])
                    nc.vector.reciprocal(rec[64:65, 512:768], accs1[64:65, :])
                    nc.sync.dma_start(rec[0:1, :], rec[64:65, :])
                    bc = wp.tile([HD, 3 * BLK], f32, tag="bc")
                    nc.gpsimd.partition_broadcast(bc[:], rec[0:1, :])
                    for i in range(3):
                        h = 3 * g + i
                        t, lo = divmod(h, 2)
                        src = (
                            accp[0:HD, i * BLK : (i + 1) * BLK]
                            if i < 2
                            else accs1[0:HD, :]
                        )
                        if lo == 0:
                            nc.vector.tensor_tensor(
                                cts[t][0:HD, :], src, bc[:, i * BLK : (i + 1) * BLK],
                                ALU.mult,
                            )
                        else:
                            stg = wp.tile([HD, BLK], bf16, tag="stg")
                            nc.vector.tensor_tensor(
                                stg[:], src, bc[:, i * BLK : (i + 1) * BLK], ALU.mult
                            )
                            nc.sync.dma_start(cts[t][HD:128, :], stg[:])

                prev = None  # (g, kc, esb)
                units_left = 3 * ext
                for g in range(NKV):
                    accs[g] = (
                        psA.tile([65, 512], f32, tag="accp", name="accp"),
                        psA.tile([65, BLK], f32, tag="accs", name="accs"),
                    )
                    for kc in range(ext):
                        esb = emit_qk(g, kc)
                        if prev is not None:
                            pg, pkc, pesb = prev
                            emit_pv(pg, pkc, pesb)
                            if pkc == ext - 1:
                                emit_norm(pg)
                        n = -(-len(filler) // units_left)  # ceil
                        pop(min(n, 3))
                        units_left -= 1
                        prev = (g, kc, esb)
                pg, pkc, pesb = prev
                emit_pv(pg, pkc, pesb)
                emit_norm(pg)
                pop(2)

                # out projection
                q0 = j * BLK
                for half in range(2):
                    h0c = half * 128
                    pa = ps1.tile([128, 512], f32, tag="ps1")
                    pb = ps1.tile([128, 64], f32, tag="ps1")
                    for t in range(5):
                        rw = 128 if t < 4 else 64
                        lhsT = cts[t][:, h0c : h0c + 128]
                        nc.tensor.matmul(
                            pa[:], lhsT, wo_sb[0:rw, t * 576 : t * 576 + 512],
                            start=(t == 0), stop=(t == 4),
                        )
                        nc.tensor.matmul(
                            pb[:], lhsT, wo_sb[0:rw, t * 576 + 512 : t * 576 + 576],
                            start=(t == 0), stop=(t == 4),
                        )
                    osb = outp.tile([128, H], f32, tag="osb")
                    nc.vector.tensor_copy(osb[:, 0:512], pa[:])
                    nc.vector.tensor_copy(osb[:, 512:576], pb[:])
                    nc.sync.dma_start(out.ap()[q0 + h0c : q0 + h0c + 128, :], osb[:])
                    pop(2)
                # anything left over still belongs before the next stage
                while filler:
                    filler.pop(0)()

            def stage0(rep):
                return xq_closure() + kv_closures(0) + q_closures(0)

            # prologue: stage-0 projections of the first rep
            for c in stage0(0):
                c()
            for rep in range(reps):
                attn(0, kv_closures(1) + q_closures(1))
                attn(1, kv_closures(2) + q_closures(2))
                attn(2, kv_closures(3) + q_closures(3))
                attn(3, stage0(rep + 1) if rep + 1 < reps else [])

    nc.compile()
    return nc


def _get_nc(reps=1):
    key = f"nc{reps}"
    if key not in _CACHED:
        _CACHED[key] = _build(reps=reps)
    return _CACHED[key]


def _make_in_maps(x, cos, sin, mask, Wq, Wk, Wv, Wo):
    import ml_dtypes

    f4 = np.float32
    bf = ml_dtypes.bfloat16
    half = HD // 2
    P = np.zeros((HD, HD), f4)
    for m in range(half):
        P[m + half, m] = -1.0
    for m in range(half, HD):
        P[m - half, m] = 1.0
    P2 = np.zeros((128, 128), f4)
    P2[0:64, 0:64] = P
    P2[64:128, 64:128] = P

    scale = np.float32(1.0 / np.sqrt(HD))
    cosT = np.ascontiguousarray(np.asarray(cos).T.astype(f4))  # [64, S]
    sinT = np.ascontiguousarray(np.asarray(sin).T.astype(f4))
    cosk2 = np.concatenate([cosT, cosT], axis=0)  # [128, S]
    sink2 = np.concatenate([sinT, sinT], axis=0)
    maskT_full = np.ascontiguousarray(np.asarray(mask)[0, 0].T.astype(f4))  # [k, q]

    def pack_planes(W, cols_out):
        C = W.shape[1]
        out_arr = np.zeros((128, 5 * cols_out), f4)
        for kt, hk in enumerate(HK):
            out_arr[0:hk, kt * cols_out : kt * cols_out + C] = W[
                kt * 128 : kt * 128 + hk, :
            ]
        return out_arr

    Wq_pk = pack_planes(np.asarray(Wq).astype(f4), 640)
    Wk_pk = pack_planes(np.asarray(Wk).astype(f4), 192)
    Wv_pk = pack_planes(np.asarray(Wv).astype(f4), 192)
    Wo_a = np.asarray(Wo).astype(f4)
    Wo_pk = np.zeros((128, 5 * 576), f4)
    for t in range(5):
        rw = 128 if t < 4 else 64
        Wo_pk[0:rw, t * 576 : (t + 1) * 576] = Wo_a[t * 128 : t * 128 + rw, :]

    x = np.asarray(x).astype(f4)
    in_maps = []
    for c in range(8):
        b = c // 2
        blocks = BLOCKS_EVEN if c % 2 == 0 else BLOCKS_ODD
        xT = np.ascontiguousarray(x[b].T)  # [H, S]
        xpad = np.zeros((640, S), f4)
        xpad[0:H, :] = xT
        planes = xpad.reshape(5, 128, S)
        xs_arr = np.zeros((4, 128, 5 * 512), f4)
        for ch in range(4):
            for kt in range(5):
                xs_arr[ch, :, kt * 512 : (kt + 1) * 512] = planes[
                    kt, :, ch * 512 : (ch + 1) * 512
                ]
        qcols = np.concatenate(
            [xpad[:, blk * BLK : (blk + 1) * BLK] for blk in blocks], axis=1
        )  # [640, 1024]
        qplanes = qcols.reshape(5, 128, 1024)
        xq_arr = np.zeros((128, 5 * 1024), f4)
        for kt in range(5):
            xq_arr[:, kt * 1024 : (kt + 1) * 1024] = qplanes[kt]

        cosq = np.concatenate(
            [cosk2[:, blk * BLK : (blk + 1) * BLK] for blk in blocks], axis=1
        ) * scale
        sinq = np.concatenate(
            [sink2[:, blk * BLK : (blk + 1) * BLK] for blk in blocks], axis=1
        ) * scale

        # masks tripled along the head axis: one multiply covers the whole
        # [128, 3*BLK] exp tile
        mskc = np.empty((128, NMASK * 3 * BLK), f4)
        for jj, blk in enumerate(blocks):
            for off in range(4):
                kc = 4 * jj + off
                sl = maskT_full[kc * KT : (kc + 1) * KT, blk * BLK : (blk + 1) * BLK]
                m01 = (sl > -1.0).astype(f4)
                for i in range(3):
                    mskc[:, (kc * 3 + i) * BLK : (kc * 3 + i + 1) * BLK] = m01

        in_maps.append(
            {
                "xs": xs_arr.astype(bf),
                "xqd": np.ascontiguousarray(xq_arr).astype(bf),
                "wq": Wq_pk.astype(bf),
                "wk": Wk_pk.astype(bf),
                "wv": Wv_pk.astype(bf),
                "wo": Wo_pk.astype(bf),
                "p2d": P2.astype(bf),
                "coskd": cosk2.astype(bf),
                "sinkd": sink2.astype(bf),
                "cosqd": np.ascontiguousarray(cosq).astype(bf),
                "sinqd": np.ascontiguousarray(sinq).astype(bf),
                "mskd": mskc.astype(bf),
            }
        )
    return in_maps


def kernel(x, cos, sin, mask, Wq, Wk, Wv, Wo, _trace=False, _trace_kwargs=None):
    from concourse import bass_utils

    in_maps = _make_in_maps(
        np.asarray(x), np.asarray(cos), np.asarray(sin), np.asarray(mask),
        np.asarray(Wq), np.asarray(Wk), np.asarray(Wv), np.asarray(Wo),
    )
    nc = _get_nc()
    kw = {}
    if _trace:
        kw["trace"] = True
        if _trace_kwargs:
            kw.update(_trace_kwargs)
    res = bass_utils.run_bass_kernel_spmd(nc, in_maps, core_ids=list(range(8)), **kw)
    out = np.empty((B, S, H), np.float32)
    for c in range(8):
        b = c // 2
        blocks = BLOCKS_EVEN if c % 2 == 0 else BLOCKS_ODD
        o = res.results[c]["out"]  # [1024, 576]
        for j, blk in enumerate(blocks):
            out[b, blk * BLK : (blk + 1) * BLK, :] = o[j * BLK : (j + 1) * BLK, :]
    if _trace:
        _CACHED["last_result"] = res
    return out


# revision 14
# speedup vs baseline: 3.6812x; 2.3914x over previous
"""Distributed GQA attention kernel for Trainium2 (8 NeuronCores).

Module: B=4, S=2048, H=576, 9 Q heads / 3 KV heads, HD=64, RoPE, causal
softmax, output projection.

Sharding: core c handles batch c//2 and four 256-row query blocks
({0,3,4,7} for even c, {1,2,5,6} for odd c) -- causal work is balanced at
36 real key-tile units per core (padded to 40 for SPMD uniformity; the
causal mask is applied from per-core mask DATA on the last 4 key-tiles of
each slot). Every core computes its batch's full K/V projection locally.

All matmul operands live at partition base 0 (PE tile-position switches
drain the array and cost ~3us each on HW). Projections still stack two
64-dim heads into 128 PSUM partitions; the upper halves are relocated to
base-0 SBUF tiles by cheap SBUF->SBUF DMAs. bf16 everywhere with f32 PSUM
accumulation. Softmax skips the max subtraction; row sums ride along as a
65th PV output row via ones columns interleaved in V. Constants load to
SBUF once at startup; the steady-state pass issues ~33 DMAs.

The attention inner loop is software-pipelined: QK(kc+1) is emitted
before PV(kc) so the PE never sits behind the exp latency, and the
projection work of the NEXT stage is chopped into closures that are
drip-fed into the PE queue between attention units (the last stage pulls
in the next repetition's stage-0 work).
"""

import sys

if "/opt/trn_rl_repo" not in sys.path:
    sys.path.insert(0, "/opt/trn_rl_repo")

import numpy as np

B, S, H = 4, 2048, 576
NH, NKV, HD = 9, 3, 64
BLK = 256           # query block rows
NBLK = S // BLK     # 8
KT = 128            # key tile rows
EXT = [4, 8, 12, 16]  # padded key-tile extent per block slot
NMASK = 16          # total masked key-tiles per core (= last-4 per slot)
BLOCKS_EVEN = [0, 3, 4, 7]
BLOCKS_ODD = [1, 2, 5, 6]
HK = [128, 128, 128, 128, 64]  # contraction tiles over H=576
NPAIR = 5           # q head pairs: (0,1),(2,3),(4,5),(6,7),(8,-)
PW = [128, 128, 128, 128, 64]  # pair widths

_CACHED = {}


def _build(reps=1):
    from concourse import bacc, bass, mybir, tile

    f32 = mybir.dt.float32
    bf16 = mybir.dt.bfloat16
    AF = mybir.ActivationFunctionType
    ALU = mybir.AluOpType

    nc = bacc.Bacc("TRN2", target_bir_lowering=False, debug=False)

    # ---- per-core inputs (bf16 unless noted) ----
    xs = nc.dram_tensor("xs", [4, 128, 5 * 512], bf16, kind="ExternalInput")
    xqd = nc.dram_tensor("xqd", [128, 5 * 1024], bf16, kind="ExternalInput")
    wq = nc.dram_tensor("wq", [128, 5 * 640], bf16, kind="ExternalInput")
    wk = nc.dram_tensor("wk", [128, 5 * 192], bf16, kind="ExternalInput")
    wv = nc.dram_tensor("wv", [128, 5 * 192], bf16, kind="ExternalInput")
    wo = nc.dram_tensor("wo", [128, 5 * 576], bf16, kind="ExternalInput")
    p2d = nc.dram_tensor("p2d", [128, 128], bf16, kind="ExternalInput")
    coskd = nc.dram_tensor("coskd", [128, S], bf16, kind="ExternalInput")
    sinkd = nc.dram_tensor("sinkd", [128, S], bf16, kind="ExternalInput")
    cosqd = nc.dram_tensor("cosqd", [128, 4 * BLK], bf16, kind="ExternalInput")
    sinqd = nc.dram_tensor("sinqd", [128, 4 * BLK], bf16, kind="ExternalInput")
    mskd = nc.dram_tensor("mskd", [128, NMASK * 3 * BLK], bf16, kind="ExternalInput")
    out = nc.dram_tensor("out", [4 * BLK, H], f32, kind="ExternalOutput")

    with tile.TileContext(nc) as tc:
        with (
            tc.tile_pool(name="consts", bufs=1) as cp,
            tc.tile_pool(name="xstream", bufs=2) as xsp,
            tc.tile_pool(name="kvres", bufs=1) as kvres,
            tc.tile_pool(name="qtp", bufs=1) as qtp,
            tc.tile_pool(name="work", bufs=2) as wp,
            tc.tile_pool(name="expp", bufs=4) as expp,
            tc.tile_pool(name="ctp", bufs=1) as ctp,
            tc.tile_pool(name="outp", bufs=2) as outp,
            tc.tile_pool(name="ps1", bufs=2, space="PSUM") as ps1,
            tc.tile_pool(name="scp", bufs=2, space="PSUM") as scp,
            tc.tile_pool(name="psA", bufs=1, space="PSUM") as psA,
        ):
            # ---- load constants (startup only) ----
            wq_sb = cp.tile([128, 5 * 640], bf16, tag="wq")
            nc.sync.dma_start(wq_sb[:], wq.ap())
            wk_sb = cp.tile([128, 5 * 192], bf16, tag="wk")
            nc.sync.dma_start(wk_sb[:], wk.ap())
            wv_sb = cp.tile([128, 5 * 192], bf16, tag="wv")
            nc.sync.dma_start(wv_sb[:], wv.ap())
            wo_sb = cp.tile([128, 5 * 576], bf16, tag="wo")
            nc.sync.dma_start(wo_sb[:], wo.ap())
            p2_sb = cp.tile([128, 128], bf16, tag="p2")
            nc.sync.dma_start(p2_sb[:], p2d.ap())
            cosk_sb = cp.tile([128, S], bf16, tag="cosk")
            nc.sync.dma_start(cosk_sb[:], coskd.ap())
            sink_sb = cp.tile([128, S], bf16, tag="sink")
            nc.sync.dma_start(sink_sb[:], sinkd.ap())
            cosq_sb = cp.tile([128, 4 * BLK], bf16, tag="cosq")
            nc.sync.dma_start(cosq_sb[:], cosqd.ap())
            sinq_sb = cp.tile([128, 4 * BLK], bf16, tag="sinq")
            nc.sync.dma_start(sinq_sb[:], sinqd.ap())
            msk_sb = cp.tile([128, NMASK * 3 * BLK], bf16, tag="msk")
            nc.sync.dma_start(msk_sb[:], mskd.ap())

            # persistent K/V/Q result tiles
            kTp2 = kvres.tile([128, S], bf16, tag="kTp2", name="kTp2")  # g0|g1
            kTg1 = kvres.tile([HD, S], bf16, tag="kTg1", name="kTg1")  # g1 @ base 0
            kTg2 = kvres.tile([HD, S], bf16, tag="kTg2", name="kTg2")  # g2
            # V: 16 key-tiles x (3 groups x 65 cols: 64 v-dims + ones)
            vch = kvres.tile([128, 16 * 195], bf16, tag="vch", name="vch")
            qT = [
                qtp.tile([128, 4 * BLK], bf16, tag=f"qT{p}", name=f"qT{p}")
                for p in range(NPAIR)
            ]
            qhi = [
                qtp.tile([HD, 4 * BLK], bf16, tag=f"qhi{p}", name=f"qhi{p}")
                for p in range(4)
            ]

            # ones columns of vch (positions g*65+64 within each 195 block)
            for st in range(16):
                vones = vch[:, st * 195 : st * 195 + 195].rearrange(
                    "p (g c) -> p g c", c=65
                )[:, :, 64:65]
                nc.vector.memset(vones, 1.0)

            # trigger the exp ACT-table load during the startup DMA wait
            warm = cp.tile([1, 1], f32, tag="warm")
            nc.scalar.activation(warm[:], p2_sb[0:1, 0:1], AF.Exp)

            holder = {}

            def xq_closure():
                def go():
                    t = qtp.tile([128, 5 * 1024], bf16, tag="xq", name="xq")
                    nc.sync.dma_start(t[:], xqd.ap())
                    holder["xq"] = t

                return [go]

            def kv_closures(ch):
                c0 = ch * 512
                cls = []
                st_h = {}

                def dma_x():
                    t = xsp.tile([128, 5 * 512], bf16, tag="xch", name=f"xch{ch}")
                    nc.sync.dma_start(t[:], xs.ap()[ch])
                    st_h["x"] = t

                cls.append(dma_x)

                def k_chain(which):
                    w = 128 if which == 0 else 64
                    xch = st_h["x"]
                    kps = ps1.tile([128, 512], f32, tag="ps1")
                    for kt, hk in enumerate(HK):
                        nc.tensor.matmul(
                            kps[0:w, :],
                            wk_sb[0:hk, kt * 192 + which * 128 : kt * 192 + which * 128 + w],
                            xch[0:hk, kt * 512 : (kt + 1) * 512],
                            start=(kt == 0),
                            stop=(kt == 4),
                        )
                    st_h[f"kps{which}"] = kps

                def k_copy(which):
                    w = 128 if which == 0 else 64
                    kraw = wp.tile([128, 512], bf16, tag="kraw", name=f"kraw{which}")
                    nc.vector.tensor_copy(kraw[0:w, :], st_h[f"kps{which}"][0:w, :])
                    st_h[f"kraw{which}"] = kraw

                def k_rot(which):
                    w = 128 if which == 0 else 64
                    rps = ps1.tile([128, 512], f32, tag="ps1")
                    nc.tensor.matmul(
                        rps[0:w, :], p2_sb[0:w, 0:w], st_h[f"kraw{which}"][0:w, :],
                        start=True, stop=True,
                    )
                    st_h[f"rps{which}"] = rps

                def k_t1(which):
                    w = 128 if which == 0 else 64
                    t1 = wp.tile([128, 512], bf16, tag="t1")
                    nc.vector.tensor_tensor(
                        t1[0:w, :], st_h[f"kraw{which}"][0:w, :],
                        cosk_sb[0:w, c0 : c0 + 512], ALU.mult,
                    )
                    st_h[f"t1{which}"] = t1

                def k_t2(which):
                    w = 128 if which == 0 else 64
                    t2 = wp.tile([128, 512], bf16, tag="t2")
                    nc.vector.tensor_tensor(
                        t2[0:w, :], st_h[f"rps{which}"][0:w, :],
                        sink_sb[0:w, c0 : c0 + 512], ALU.mult,
                    )
                    st_h[f"t2{which}"] = t2

                def k_add(which):
                    w = 128 if which == 0 else 64
                    dst = kTp2 if which == 0 else kTg2
                    nc.vector.tensor_tensor(
                        dst[0:w, c0 : c0 + 512],
                        st_h[f"t1{which}"][0:w, :],
                        st_h[f"t2{which}"][0:w, :],
                        ALU.add,
                    )
                    if which == 0:
                        # relocate g1 to a base-0 tile (PE needs uniform
                        # operand partition bases)
                        nc.sync.dma_start(
                            kTg1[:, c0 : c0 + 512], kTp2[64:128, c0 : c0 + 512]
                        )

                for which in range(2):
                    cls.append(lambda w_=which: k_chain(w_))
                    cls.append(lambda w_=which: k_copy(w_))
                    cls.append(lambda w_=which: k_rot(w_))
                    cls.append(lambda w_=which: k_t1(w_))
                    cls.append(lambda w_=which: k_t2(w_))
                    cls.append(lambda w_=which: k_add(w_))

                def v_chain(st4):
                    xch = st_h["x"]
                    vps = ps1.tile([128, 192], f32, tag="ps1")
                    for kt, hk in enumerate(HK):
                        nc.tensor.matmul(
                            vps[:],
                            xch[0:hk, kt * 512 + st4 * 128 : kt * 512 + (st4 + 1) * 128],
                            wv_sb[0:hk, kt * 192 : (kt + 1) * 192],
                            start=(kt == 0),
                            stop=(kt == 4),
                        )
                    st_h[f"vps{st4}"] = vps

                def v_copy(st4):
                    st = ch * 4 + st4
                    dst = vch[:, st * 195 : st * 195 + 195].rearrange(
                        "p (g c) -> p g c", c=65
                    )[:, :, 0:64]
                    src = st_h[f"vps{st4}"][:].rearrange("p (g c) -> p g c", c=64)
                    nc.vector.tensor_copy(dst, src)

                for st4 in range(4):
                    cls.append(lambda s_=st4: v_chain(s_))
                    cls.append(lambda s_=st4: v_copy(s_))
                return cls

            def q_closures(j):
                cls = []
                st_h = {}

                def q_chain(p):
                    pw = PW[p]
                    xq_t = holder["xq"]
                    qps = ps1.tile([128, BLK], f32, tag="ps1")
                    for kt, hk in enumerate(HK):
                        nc.tensor.matmul(
                            qps[0:pw, :],
                            wq_sb[0:hk, kt * 640 + p * 128 : kt * 640 + p * 128 + pw],
                            xq_t[0:hk, kt * 1024 + j * BLK : kt * 1024 + (j + 1) * BLK],
                            start=(kt == 0),
                            stop=(kt == 4),
                        )
                    st_h[f"qps{p}"] = qps

                def q_copy(p):
                    pw = PW[p]
                    qraw = wp.tile([128, BLK], bf16, tag="qraw")
                    nc.vector.tensor_copy(qraw[0:pw, :], st_h[f"qps{p}"][0:pw, :])
                    st_h[f"qraw{p}"] = qraw

                def q_rot(p):
                    pw = PW[p]
                    rps = ps1.tile([128, BLK], f32, tag="ps1")
                    nc.tensor.matmul(
                        rps[0:pw, :], p2_sb[0:pw, 0:pw], st_h[f"qraw{p}"][0:pw, :],
                        start=True, stop=True,
                    )
                    st_h[f"rq{p}"] = rps

                def q_t1(p):
                    pw = PW[p]
                    tq1 = wp.tile([128, BLK], bf16, tag="tq1")
                    nc.vector.tensor_tensor(
                        tq1[0:pw, :], st_h[f"qraw{p}"][0:pw, :],
                        cosq_sb[0:pw, j * BLK : (j + 1) * BLK], ALU.mult,
                    )
                    st_h[f"tq1{p}"] = tq1

                def q_t2(p):
                    pw = PW[p]
                    tq2 = wp.tile([128, BLK], bf16, tag="tq2")
                    nc.vector.tensor_tensor(
                        tq2[0:pw, :], st_h[f"rq{p}"][0:pw, :],
                        sinq_sb[0:pw, j * BLK : (j + 1) * BLK], ALU.mult,
                    )
                    st_h[f"tq2{p}"] = tq2

                def q_add(p):
                    pw = PW[p]
                    nc.vector.tensor_tensor(
                        qT[p][0:pw, j * BLK : (j + 1) * BLK],
                        st_h[f"tq1{p}"][0:pw, :],
                        st_h[f"tq2{p}"][0:pw, :],
                        ALU.add,
                    )
                    if pw == 128:
                        nc.sync.dma_start(
                            qhi[p][:, j * BLK : (j + 1) * BLK],
                            qT[p][64:128, j * BLK : (j + 1) * BLK],
                        )

                for p in range(NPAIR):
                    cls.append(lambda p_=p: q_chain(p_))
                    cls.append(lambda p_=p: q_copy(p_))
                    cls.append(lambda p_=p: q_rot(p_))
                    cls.append(lambda p_=p: q_t1(p_))
                    cls.append(lambda p_=p: q_t2(p_))
                    cls.append(lambda p_=p: q_add(p_))
                return cls

            def attn(j, filler):
                def pop(n=1):
                    for _ in range(n):
                        if not filler:
                            return
                        filler.pop(0)()

                ext = EXT[j]
                cts = [
                    ctp.tile([128, BLK], bf16, tag=f"ct{t}", name=f"ct{t}")
                    for t in range(4)
                ]
                cts.append(ctp.tile([HD, BLK], bf16, tag="ct4", name="ct4"))
                KSRC = (None, kTg1, kTg2)

                accs = {}

                def emit_qk(g, kc):
                    sps = scp.tile([KT, 3 * BLK], f32, tag="sc")
                    h0 = 3 * g
                    for i in range(3):
                        h = h0 + i
                        p, half = divmod(h, 2)
                        if g == 0:
                            lhsT = kTp2[0:64, kc * KT : (kc + 1) * KT]
                        else:
                            lhsT = KSRC[g][:, kc * KT : (kc + 1) * KT]
                        if half == 0:
                            rhs = qT[p][0:64, j * BLK : (j + 1) * BLK]
                        else:
                            rhs = qhi[p][:, j * BLK : (j + 1) * BLK]
                        nc.tensor.matmul(
                            sps[:, i * BLK : (i + 1) * BLK], lhsT, rhs,
                            start=True, stop=True,
                        )
                    esb = expp.tile([KT, 3 * BLK], bf16, tag="exp")
                    nc.scalar.activation(esb[:], sps[:], AF.Exp)
                    if kc >= ext - 4:
                        nc.vector.tensor_tensor(
                            esb[:], esb[:],
                            msk_sb[:, kc * 3 * BLK : (kc + 1) * 3 * BLK], ALU.mult,
                        )
                    return esb

                def emit_pv(g, kc, esb):
                    accp, accs1 = accs[g]
                    vslice = vch[:, kc * 195 + g * 65 : kc * 195 + g * 65 + 65]
                    nc.tensor.matmul(
                        accp[:], vslice, esb[:, 0:512],
                        start=(kc == 0), stop=(kc == ext - 1),
                    )
                    nc.tensor.matmul(
                        accs1[:], vslice, esb[:, 512:768],
                        start=(kc == 0), stop=(kc == ext - 1),
                    )

                def emit_norm(g):
                    accp, accs1 = accs[g]
                    # evacuate PSUM fast (Act copies) so the accumulator bank
                    # frees for the next group; normalize from SBUF
                    acc_sb = wp.tile([65, 3 * BLK], f32, tag="accsb")
                    nc.scalar.activation(acc_sb[:, 0:512], accp[:], AF.Copy)
                    nc.scalar.activation(acc_sb[:, 512:768], accs1[:], AF.Copy)
                    rec = wp.tile([65, 3 * BLK], f32, tag="rec")
                    nc.vector.reciprocal(rec[64:65, :], acc_sb[64:65, :])
                    nc.sync.dma_start(rec[0:1, :], rec[64:65, :])
                    bc = wp.tile([HD, 3 * BLK], f32, tag="bc")
                    nc.gpsimd.partition_broadcast(bc[:], rec[0:1, :])
                    for i in range(3):
                        h = 3 * g + i
                        t, lo = divmod(h, 2)
                        src = acc_sb[0:HD, i * BLK : (i + 1) * BLK]
                        if lo == 0:
                            nc.vector.tensor_tensor(
                                cts[t][0:HD, :], src, bc[:, i * BLK : (i + 1) * BLK],
                                ALU.mult,
                            )
                        else:
                            stg = wp.tile([HD, BLK], bf16, tag="stg")
                            nc.vector.tensor_tensor(
                                stg[:], src, bc[:, i * BLK : (i + 1) * BLK], ALU.mult
                            )
                            nc.sync.dma_start(cts[t][HD:128, :], stg[:])

                prev = None  # (g, kc, esb)
                units_left = 3 * ext
                for g in range(NKV):
                    accs[g] = (
                        psA.tile([65, 512], f32, tag="accp", name="accp"),
                        psA.tile([65, BLK], f32, tag="accs", name="accs"),
                    )
                    for kc in range(ext):
                        esb = emit_qk(g, kc)
                        if prev is not None:
                            pg, pkc, pesb = prev
                            emit_pv(pg, pkc, pesb)
                            if pkc == ext - 1:
                                emit_norm(pg)
                        n = -(-len(filler) // units_left)  # ceil
                        pop(min(n, 3))
                        units_left -= 1
                        prev = (g, kc, esb)
                pg, pkc, pesb = prev
                emit_pv(pg, pkc, pesb)
                emit_norm(pg)
                pop(2)

                # out projection
                q0 = j * BLK
                for half in range(2):
                    h0c = half * 128
                    pa = ps1.tile([128, 512], f32, tag="ps1")
                    pb = ps1.tile([128, 64], f32, tag="ps1")
                    for t in range(5):
                        rw = 128 if t < 4 else 64
                        lhsT = cts[t][:, h0c : h0c + 128]
                        nc.tensor.matmul(
                            pa[:], lhsT, wo_sb[0:rw, t * 576 : t * 576 + 512],
                            start=(t == 0), stop=(t == 4),
                        )
                        nc.tensor.matmul(
                            pb[:], lhsT, wo_sb[0:rw, t * 576 + 512 : t * 576 + 576],
                            start=(t == 0), stop=(t == 4),
                        )
                    osb = outp.tile([128, H], f32, tag="osb")
                    nc.vector.tensor_copy(osb[:, 0:512], pa[:])
                    nc.vector.tensor_copy(osb[:, 512:576], pb[:])
                    nc.sync.dma_start(out.ap()[q0 + h0c : q0 + h0c + 128, :], osb[:])
                    pop(2)
                # anything left over still belongs before the next stage
                while filler:
                    filler.pop(0)()

            def stage0(rep):
                return xq_closure() + kv_closures(0) + q_closures(0)

            # prologue: stage-0 projections of the first rep
            for c in stage0(0):
                c()
            for rep in range(reps):
                attn(0, kv_closures(1) + q_closures(1))
                attn(1, kv_closures(2) + q_closures(2))
                attn(2, kv_closures(3) + q_closures(3))
                attn(3, stage0(rep + 1) if rep + 1 < reps else [])

    nc.compile()
    return nc


def _get_nc(reps=1):
    key = f"nc{reps}"
    if key not in _CACHED:
        _CACHED[key] = _build(reps=reps)
    return _CACHED[key]


def _make_in_maps(x, cos, sin, mask, Wq, Wk, Wv, Wo):
    import ml_dtypes

    f4 = np.float32
    bf = ml_dtypes.bfloat16
    half = HD // 2
    P = np.zeros((HD, HD), f4)
    for m in range(half):
        P[m + half, m] = -1.0
    for m in range(half, HD):
        P[m - half, m] = 1.0
    P2 = np.zeros((128, 128), f4)
    P2[0:64, 0:64] = P
    P2[64:128, 64:128] = P

    scale = np.float32(1.0 / np.sqrt(HD))
    cosT = np.ascontiguousarray(np.asarray(cos).T.astype(f4))  # [64, S]
    sinT = np.ascontiguousarray(np.asarray(sin).T.astype(f4))
    cosk2 = np.concatenate([cosT, cosT], axis=0)  # [128, S]
    sink2 = np.concatenate([sinT, sinT], axis=0)
    maskT_full = np.ascontiguousarray(np.asarray(mask)[0, 0].T.astype(f4))  # [k, q]

    def pack_planes(W, cols_out):
        C = W.shape[1]
        out_arr = np.zeros((128, 5 * cols_out), f4)
        for kt, hk in enumerate(HK):
            out_arr[0:hk, kt * cols_out : kt * cols_out + C] = W[
                kt * 128 : kt * 128 + hk, :
            ]
        return out_arr

    Wq_pk = pack_planes(np.asarray(Wq).astype(f4), 640)
    Wk_pk = pack_planes(np.asarray(Wk).astype(f4), 192)
    Wv_pk = pack_planes(np.asarray(Wv).astype(f4), 192)
    Wo_a = np.asarray(Wo).astype(f4)
    Wo_pk = np.zeros((128, 5 * 576), f4)
    for t in range(5):
        rw = 128 if t < 4 else 64
        Wo_pk[0:rw, t * 576 : (t + 1) * 576] = Wo_a[t * 128 : t * 128 + rw, :]

    x = np.asarray(x).astype(f4)
    in_maps = []
    for c in range(8):
        b = c // 2
        blocks = BLOCKS_EVEN if c % 2 == 0 else BLOCKS_ODD
        xT = np.ascontiguousarray(x[b].T)  # [H, S]
        xpad = np.zeros((640, S), f4)
        xpad[0:H, :] = xT
        planes = xpad.reshape(5, 128, S)
        xs_arr = np.zeros((4, 128, 5 * 512), f4)
        for ch in range(4):
            for kt in range(5):
                xs_arr[ch, :, kt * 512 : (kt + 1) * 512] = planes[
                    kt, :, ch * 512 : (ch + 1) * 512
                ]
        qcols = np.concatenate(
            [xpad[:, blk * BLK : (blk + 1) * BLK] for blk in blocks], axis=1
        )  # [640, 1024]
        qplanes = qcols.reshape(5, 128, 1024)
        xq_arr = np.zeros((128, 5 * 1024), f4)
        for kt in range(5):
            xq_arr[:, kt * 1024 : (kt + 1) * 1024] = qplanes[kt]

        cosq = np.concatenate(
            [cosk2[:, blk * BLK : (blk + 1) * BLK] for blk in blocks], axis=1
        ) * scale
        sinq = np.concatenate(
            [sink2[:, blk * BLK : (blk + 1) * BLK] for blk in blocks], axis=1
        ) * scale

        # masks tripled along the head axis: one multiply covers the whole
        # [128, 3*BLK] exp tile
        mskc = np.empty((128, NMASK * 3 * BLK), f4)
        for jj, blk in enumerate(blocks):
            for off in range(4):
                kc = 4 * jj + off
                sl = maskT_full[kc * KT : (kc + 1) * KT, blk * BLK : (blk + 1) * BLK]
                m01 = (sl > -1.0).astype(f4)
                for i in range(3):
                    mskc[:, (kc * 3 + i) * BLK : (kc * 3 + i + 1) * BLK] = m01

        in_maps.append(
            {
                "xs": xs_arr.astype(bf),
                "xqd": np.ascontiguousarray(xq_arr).astype(bf),
                "wq": Wq_pk.astype(bf),
                "wk": Wk_pk.astype(bf),
                "wv": Wv_pk.astype(bf),
                "wo": Wo_pk.astype(bf),
                "p2d": P2.astype(bf),
                "coskd": cosk2.astype(bf),
                "sinkd": sink2.astype(bf),
                "cosqd": np.ascontiguousarray(cosq).astype(bf),
                "sinqd": np.ascontiguousarray(sinq).astype(bf),
                "mskd": mskc.astype(bf),
            }
        )
    return in_maps


def kernel(x, cos, sin, mask, Wq, Wk, Wv, Wo, _trace=False, _trace_kwargs=None):
    from concourse import bass_utils

    in_maps = _make_in_maps(
        np.asarray(x), np.asarray(cos), np.asarray(sin), np.asarray(mask),
        np.asarray(Wq), np.asarray(Wk), np.asarray(Wv), np.asarray(Wo),
    )
    nc = _get_nc()
    kw = {}
    if _trace:
        kw["trace"] = True
        if _trace_kwargs:
            kw.update(_trace_kwargs)
    res = bass_utils.run_bass_kernel_spmd(nc, in_maps, core_ids=list(range(8)), **kw)
    out = np.empty((B, S, H), np.float32)
    for c in range(8):
        b = c // 2
        blocks = BLOCKS_EVEN if c % 2 == 0 else BLOCKS_ODD
        o = res.results[c]["out"]  # [1024, 576]
        for j, blk in enumerate(blocks):
            out[b, blk * BLK : (blk + 1) * BLK, :] = o[j * BLK : (j + 1) * BLK, :]
    if _trace:
        _CACHED["last_result"] = res
    return out
